# revision 1
# baseline (speedup 1.0000x reference)
"""AttentionBlock (GroupNorm + 1x1-conv QKV self-attention + out-proj + residual)
as a distributed Bass kernel on 8 TRN2 NeuronCores.

Sharding: fully data-parallel, zero collectives.
  core = 2*b + half   (b in 0..3 batch element, half in 0..1 query-row half)
Each core receives x COLUMN-ROTATED so its own 2048 query columns come
first (GN stats, K/V projections, softmax and PV sums are permutation-
invariant over key order, so ONE load serves stats, K/V over all 4096
keys, and Q/residual over the first 2048 columns). x ships as fp16
(4MB instead of 8): stats accumulate in fp32 regardless of input
dtype, projections consume an fp8 cast anyway, and the fp16 residual
base costs ~1e-4 rel err against a 2e-2 gate — halving the dominant
HBM load that gates everything.
  - x streams on the SP DMA queue (7 chunks; 1 rides ScalarE's queue so
    the aggregate sits at the ~360GB/s HBM roofline); weights ride the
    ScalarE queue at t=0, small consts the Pool queue — the exp-critical
    ScalarE instruction stream itself stays DMA-free once compute starts.
    A dummy Sqrt at t=0 preloads the sqrt/identity activation table so
    the GN-stats chain's real Sqrt costs no table load. The GroupNorm
    chain runs per channel-chunk, incrementally, as each chunk's
    columns land — only the last chunk's narrow chain (and one w8
    scaling per engine) sits in the post-load critical tail.
  - GroupNorm affine h = x*A + B is FOLDED into the projections: q/k/v
    weights are scaled by 64*A on device (fp8e4; 64 keeps the ~0.02
    weights in fp8 normal range, divided back out in the epilogues) and
    B is folded into the projection biases with tiny matmuls, so the
    projections consume raw fp8-cast x directly.
  - Projections are FUSED into the first attention sweep: the ns=0
    query block's S/exp/PV ride along the k/v projection loop, so the
    ScalarE exp stream starts as soon as the first k/q tiles exist.
  - All matmuls fp8e4 DoubleRow (2x PE throughput), fp32 PSUM accum.
  - attention in transposed layout: s_T[m, n] = sum_c k[c,m] q[c,n]
    -> exp on ScalarE (fp8 out). For ns>=1 the S psums are [P,2,512]
    2-bank tiles so ONE activation covers a 1024-col pair (halves the
    ScalarE instruction+access overhead, the critical path). exp output
    goes to a per-ns retained E8 buffer; PV runs as a rolling 2-sweep
    (e4 0,1 accumulate during the m-loop in 2 PSUM banks; e4 2,3 replay
    the retained E8 right after, interleaved into the next ns's m-loop)
    so S-pairs(4) + PV(2) + Z(1) + out-proj(1) fit the 8 PSUM banks.
    ns=0 defers BOTH its PV and Z to retained-E8 sweeps in its finalize,
    so region 1 spends all 8 banks on the projection pipeline (4) and
    S pairs (4) - the projection epilogue ring never throttles PE.
    The softmax denominator Z is a DoubleRow ones-matmul riding the
    m-loop. Normalization and the v-bias are deferred past the PV
    matmul (osb = PV/Z + bv, exact since softmax weights sum to 1),
    then fp8 out-proj (weights pre-scaled x64 on host), +bias +residual
    in fp32.
  - No on-chip transposes anywhere.
"""

import os
import sys

import numpy as np

for p in ("/opt/trn_rl_repo", "/opt/pypackages"):
    if p not in sys.path:
        sys.path.append(p)

import ml_dtypes

import concourse.bass as bass
import concourse.bacc as bacc
import concourse.tile as tile
from concourse import mybir
from concourse.bass import ts
from concourse.bass_utils import run_bass_kernel_spmd

F32 = mybir.dt.float32
F16 = mybir.dt.float16
BF16 = mybir.dt.bfloat16
FP8 = mybir.dt.float8e4
AF = mybir.ActivationFunctionType
OP = mybir.AluOpType

C = 512
N = 4096
NHALF = 2048
P = 128
CCH = C // P          # 4 channel chunks
NB = N // 512         # 8 column blocks of 512
NBH = NHALF // 512    # 4
MC = N // P           # 32 key chunks of 128
NPAIR = MC // 2       # 16 m-chunk pairs
EPS = 1e-5
SCALE = C ** -0.5
DR = mybir.MatmulPerfMode.DoubleRow
INV64 = 1.0 / 64.0

LAST_EXEC_TIME_NS = None

_CACHED_NC = None
_last_in_maps = None


def build_nc():
    nc = bacc.Bacc(None, target_bir_lowering=False)

    x_p = nc.declare_dram_parameter("xr", [CCH, P, N], F16, isOutput=False)
    wq_p = nc.declare_dram_parameter("wqT", [P, CCH, C], BF16, isOutput=False)
    wk_p = nc.declare_dram_parameter("wkT", [P, CCH, C], BF16, isOutput=False)
    wv_p = nc.declare_dram_parameter("wvT", [P, CCH, C], BF16, isOutput=False)
    wo_p = nc.declare_dram_parameter("wo8", [P, CCH, C], FP8, isOutput=False)
    bq_p = nc.declare_dram_parameter("bq", [P, CCH], F32, isOutput=False)
    bk_p = nc.declare_dram_parameter("bk", [P, CCH], F32, isOutput=False)
    bo_p = nc.declare_dram_parameter("bo", [P, CCH], F32, isOutput=False)
    bv_p = nc.declare_dram_parameter("bvc", [P, CCH], F32, isOutput=False)
    gnw_p = nc.declare_dram_parameter("gnw", [P, CCH], F32, isOutput=False)
    gnb_p = nc.declare_dram_parameter("gnb", [P, CCH], F32, isOutput=False)
    ones8_p = nc.declare_dram_parameter("ones8", [P, 2, P], FP8, isOutput=False)
    gm_p = nc.declare_dram_parameter("gmat", [P, P], F32, isOutput=False)
    out_p = nc.declare_dram_parameter("out", [CCH, P, NHALF], F16, isOutput=True)

    with tile.TileContext(nc) as tc:
        with tc.tile_pool(name="singles", bufs=1) as singles:
            k_t = singles.tile([P, CCH, N], FP8)
            q_t = singles.tile([P, CCH, NHALF], FP8)
            vT_t = singles.tile([P, MC, C], FP8)
            xb_t = singles.tile([P, CCH, NHALF], F32)
            A_t = singles.tile([P, CCH], F32)
            B_t = singles.tile([P, CCH], F32)
            B16_t = singles.tile([P, CCH], BF16)
            w_q = singles.tile([P, CCH, C], BF16)
            w_k = singles.tile([P, CCH, C], BF16)
            w_v = singles.tile([P, CCH, C], BF16)
            w8o = singles.tile([P, CCH, C], FP8)
            bq2_t = singles.tile([P, CCH], F32)
            bk2_t = singles.tile([P, CCH], F32)
            bv2c_t = singles.tile([P, CCH], F32)
            bq_t = singles.tile([P, CCH], F32)
            bk_t = singles.tile([P, CCH], F32)
            bo_t = singles.tile([P, CCH], F32)
            bv_t = singles.tile([P, CCH], F32)
            gnw_t = singles.tile([P, CCH], F32)
            gnb_t = singles.tile([P, CCH], F32)
            ones8_t = singles.tile([P, 2, P], FP8)
            gm_t = singles.tile([P, P], F32)
            eps_t = singles.tile([P, 1], F32)
            zero_t = singles.tile([P, 1], F32)
            sq_t = singles.tile([P, 1], F32)
            nc.vector.memset(eps_t, EPS)
            nc.vector.memset(zero_t, 0.0)
            # dummy: preloads the sqrt/identity act table at t=0 so the
            # GN-stats chain's real Sqrt doesn't eat a 1.3us table load
            nc.scalar.activation(out=sq_t, in_=eps_t, func=AF.Sqrt)

            # ScalarE (exp) is the busiest engine: keep its queue DMA-free.
            # Tiny latency-critical consts lead the SP queue (x follows);
            # weights ride DVE's queue (done before bn_stats need it);
            # the rest ride Pool's queue ahead of the fp8 casts.
            nc.gpsimd.dma_start(out=bo_t, in_=bo_p[:])
            nc.gpsimd.dma_start(out=gm_t, in_=gm_p[:])
            nc.scalar.dma_start(out=w_q, in_=wq_p[:])
            nc.scalar.dma_start(out=w_k, in_=wk_p[:])
            nc.scalar.dma_start(out=w_v, in_=wv_p[:])
            nc.scalar.dma_start(out=w8o, in_=wo_p[:])
            nc.gpsimd.dma_start(out=bq_t, in_=bq_p[:])
            nc.gpsimd.dma_start(out=bk_t, in_=bk_p[:])
            nc.gpsimd.dma_start(out=bv_t, in_=bv_p[:])
            nc.gpsimd.dma_start(out=gnw_t, in_=gnw_p[:])
            nc.gpsimd.dma_start(out=gnb_t, in_=gnb_p[:])
            nc.gpsimd.dma_start(out=ones8_t, in_=ones8_p[:])

            # fp8 x + fp8 GN-scaled weights, alive through the projections
            with tc.tile_pool(name="xcast", bufs=1) as xcast:
                xb16 = xcast.tile([P, CCH, N], FP8)
                w8q = xcast.tile([P, CCH, C], FP8)
                w8k = xcast.tile([P, CCH, C], FP8)
                w8v = xcast.tile([P, CCH, C], FP8)
                A64_t = xcast.tile([P, CCH], F32)

                # ---------- Phase A: GroupNorm statistics + weight folding --
                with (
                    tc.tile_pool(name="astat", bufs=4) as statp,
                    tc.tile_pool(name="aload", bufs=4) as aload,
                    tc.tile_pool(name="apsum", bufs=2, space="PSUM") as app,
                ):
                    mvall = statp.tile([P, CCH, 2], F32, tag="mvall")
                    for ci in range(CCH):
                        st6 = statp.tile([P, NB, 6], F32, tag="st6")
                        for nq in range(2):  # two 2048-wide loads per chunk
                            xt = aload.tile([P, 2048], F16, tag="xt")
                            if (ci, nq) == (0, 0):
                                # first chunk in 4 sub-DMAs (same wire time)
                                # so the serial BNStats stream — the load-
                                # phase floor — starts ~1.2us earlier
                                for sb in range(4):
                                    nc.sync.dma_start(
                                        out=xt[:, ts(sb, 512)],
                                        in_=x_p[ci, :, ts(sb, 512)],
                                    )
                                    nc.vector.bn_stats(
                                        out=st6[:, sb, :],
                                        in_=xt[:, ts(sb, 512)],
                                    )
                            else:
                                xq = (
                                    nc.scalar
                                    if (ci, nq) == (3, 1)
                                    else nc.sync
                                )
                                xq.dma_start(
                                    out=xt, in_=x_p[ci, :, ts(nq, 2048)]
                                )
                                for sb in range(4):
                                    nc.vector.bn_stats(
                                        out=st6[:, nq * 4 + sb, :],
                                        in_=xt[:, ts(sb, 512)],
                                    )
                            # cast on GpSimd (idle during phase A)
                            nc.gpsimd.tensor_copy(
                                out=xb16[:, ci, ts(nq, 2048)], in_=xt
                            )
                            if nq == 0:
                                # my-half residual base x + bo on ScalarE
                                # (Identity is in the sqrt table; ScalarE
                                # has slack during the load phase while
                                # DVE carries bn_stats)
                                nc.scalar.activation(
                                    out=xb_t[:, ci, :],
                                    in_=xt,
                                    func=AF.Identity,
                                    bias=bo_t[:, ci : ci + 1],
                                )
                        nc.vector.bn_aggr(out=mvall[:, ci, :], in_=st6)

                        # Incremental per-chunk GN chain: this chunk's group
                        # reduce -> rstd -> A64/B16 -> w8 scaling runs as
                        # soon as ITS columns are loaded, so after the LAST
                        # x chunk only one narrow chain remains in the tail
                        # (the batched version put all 12 w8 scalings there)
                        rsall = statp.tile(
                            [P, 1, 2], F32, tag="rsall", name=f"rs_{ci}"
                        )
                        nc.vector.tensor_mul(
                            out=rsall[:, :, 1:2],
                            in0=mvall[:, ci : ci + 1, 0:1],
                            in1=mvall[:, ci : ci + 1, 0:1],
                        )
                        nc.vector.tensor_add(
                            out=rsall[:, :, 1:2],
                            in0=rsall[:, :, 1:2],
                            in1=mvall[:, ci : ci + 1, 1:2],
                        )
                        nc.vector.tensor_copy(
                            out=rsall[:, :, 0:1], in_=mvall[:, ci : ci + 1, 0:1]
                        )
                        # single-matmul group reduce: G = ind@ind2 is the
                        # block-diagonal 1/16 group-averaging projector, so
                        # the two-matmul round trip (and its PSUM->SBUF hop)
                        # collapses to one mm on the chain's critical path
                        rps = app.tile(
                            [P, 1, 2], F32, tag="r", bufs=1, name=f"r_{ci}"
                        )
                        nc.tensor.matmul(
                            rps, lhsT=gm_t, rhs=rsall, start=True, stop=True
                        )
                        gm = statp.tile([P, 1], F32, tag="gmall", name=f"gm_{ci}")
                        gv = statp.tile([P, 1], F32, tag="gvall", name=f"gv_{ci}")
                        nc.vector.tensor_copy(out=gm, in_=rps[:, :, 0:1])
                        nc.vector.tensor_mul(out=gv, in0=gm, in1=gm)
                        nc.vector.tensor_sub(out=gv, in0=rps[:, :, 1:2], in1=gv)
                        # rstd = 1/sqrt(var + eps)
                        nc.scalar.activation(out=gv, in_=gv, func=AF.Sqrt, bias=eps_t)
                        nc.vector.reciprocal(out=gv, in_=gv)
                        # A64 = 64*rstd*gnw in one op; B16 written directly
                        # by the subtract (bf16 out)
                        nc.vector.scalar_tensor_tensor(
                            out=A64_t[:, ci : ci + 1],
                            in0=gv,
                            scalar=64.0,
                            in1=gnw_t[:, ci : ci + 1],
                            op0=OP.mult,
                            op1=OP.mult,
                        )
                        nc.vector.tensor_mul(
                            out=A_t[:, ci : ci + 1], in0=gv, in1=gnw_t[:, ci : ci + 1]
                        )
                        nc.vector.tensor_mul(out=gm, in0=gm, in1=A_t[:, ci : ci + 1])
                        nc.vector.tensor_sub(
                            out=B16_t[:, ci : ci + 1],
                            in0=gnb_t[:, ci : ci + 1],
                            in1=gm,
                        )
                        for wi, (wt, w8) in enumerate(
                            ((w_q, w8q), (w_k, w8k), (w_v, w8v))
                        ):
                            eng = nc.gpsimd
                            eng.tensor_scalar_mul(
                                out=w8[:, ci, :],
                                in0=wt[:, ci, :],
                                scalar1=A64_t[:, ci : ci + 1],
                            )

                    # dummy: swap ScalarE to the exp/identity table NOW
                    # (after the last real Sqrt; ScalarE is idle here, and
                    # identity epilogues + all exps then run with zero
                    # further table loads)
                    nc.scalar.activation(out=sq_t, in_=eps_t, func=AF.Exp)

                    # Fold B into projection biases:
                    #   bq2[o] = bq[o] + sum_c wqT[c,o] * B[c]   (same for bk2)
                    #   bv2c[e] = bv[e] + sum_c wvT[c,e] * B[c]
                    for (wt, b_in, b_out) in ((w_k, bk_t, bk2_t), (w_q, bq_t, bq2_t)):
                        for oj in range(CCH):
                            bc = app.tile([P, 1], F32, tag="bc", bufs=2)
                            for ci in range(CCH):
                                nc.tensor.matmul(
                                    bc,
                                    lhsT=wt[:, ci, ts(oj, P)],
                                    rhs=B16_t[:, ci : ci + 1],
                                    start=(ci == 0),
                                    stop=(ci == CCH - 1),
                                )
                            nc.vector.tensor_add(
                                out=b_out[:, oj : oj + 1],
                                in0=bc,
                                in1=b_in[:, oj : oj + 1],
                            )
                    for e4 in range(CCH):
                        bc = app.tile([P, 1], F32, tag="bc", bufs=2)
                        for ci in range(CCH):
                            nc.tensor.matmul(
                                bc,
                                lhsT=w_v[:, ci, ts(e4, P)],
                                rhs=B16_t[:, ci : ci + 1],
                                start=(ci == 0),
                                stop=(ci == CCH - 1),
                            )
                        nc.vector.tensor_add(
                            out=bv2c_t[:, e4 : e4 + 1],
                            in0=bc,
                            in1=bv_t[:, e4 : e4 + 1],
                        )

                # ---------- Fused projections + attention -------------------
                with (
                    tc.tile_pool(name="att", bufs=4) as attp,
                    tc.tile_pool(name="fin", bufs=3) as finp,
                    tc.tile_pool(name="e8big", bufs=2) as e8bp,
                ):
                    ppp_cell = [None]  # out-proj PSUM pool, opened in region 2
                    zps_cell = [None]  # softmax-Z PSUM pool, opened in region 2
                    ozp_cell = [None]  # PV PSUM pool, opened in region 2

                    # GPSIMD cannot read PSUM (BIR verifier): PSUM epilogues
                    # go on DVE (and ScalarE before the exp stream starts);
                    # v epilogues bounce PSUM->SBUF over the idle SP DMA
                    # queue so GpSimd can do the scale+cast from SBUF.
                    def proj_epi(out_ap, psum, scalar2, on_act):
                        if on_act:
                            nc.scalar.activation(
                                out=out_ap,
                                in_=psum,
                                func=AF.Identity,
                                bias=scalar2 if scalar2 is not None else zero_t,
                                scale=INV64,
                            )
                        elif scalar2 is None:
                            nc.vector.tensor_scalar_mul(
                                out=out_ap, in0=psum, scalar1=INV64
                            )
                        else:
                            nc.vector.tensor_scalar(
                                out=out_ap,
                                in0=psum,
                                scalar1=INV64,
                                scalar2=scalar2,
                                op0=OP.mult,
                                op1=OP.add,
                            )

                    def mk_pv1(ops, a, rhs):
                        # PV sweep 1: e4 0,1 accumulate during the m-loop
                        for e4 in range(2):
                            nc.tensor.matmul(
                                ops[e4],
                                lhsT=vT_t[:, 2 * a : 2 * a + 2, ts(e4, P)],
                                rhs=rhs,
                                start=(a == 0),
                                stop=(a == NPAIR - 1),
                                perf_mode=DR,
                            )

                    def mk_z(zps, a, rhs):
                        nc.tensor.matmul(
                            zps,
                            lhsT=ones8_t,
                            rhs=rhs,
                            start=(a == 0),
                            stop=(a == NPAIR - 1),
                            perf_mode=DR,
                        )

                    def finalize(ns, ops, zps, E8ns, s2pool=None):
                        """recip+osb01, PV sweep 2 (from retained E8), osb23,
                        out-proj, residual, store. Emitted right after a few
                        lookahead pairs of the NEXT ns so ScalarE stays fed.
                        For the last ns, s2pool supplies fresh PSUM banks so
                        the PV sweep runs concurrently with the recip chain."""
                        rz = attp.tile([P, 512], F32, tag="rz", name=f"rz{ns}")
                        osb = attp.tile([P, CCH, 512], FP8, tag="osb", name=f"ob{ns}")
                        osf = attp.tile([P, 2, 512], F32, tag="osf", name=f"of{ns}")
                        ozp = ozp_cell[0]
                        if zps is None:
                            # ns=0 ran without Z/PV banks (region-1 PSUM is
                            # all projection+S pipeline): recompute both here
                            # from the retained exps
                            zps = zps_cell[0].tile([P, 512], F32, tag="z", name="z0")
                            for a in range(NPAIR):
                                mk_z(zps, a, E8ns[:, 2 * a : 2 * a + 2, :])
                        if ops is None:
                            ops = [
                                ozp.tile([P, 512], F32, tag="oz", name=f"o0s1_{e4}")
                                for e4 in range(2)
                            ]
                            for a in range(NPAIR):
                                for i in range(2):
                                    nc.tensor.matmul(
                                        ops[i],
                                        lhsT=vT_t[:, 2 * a : 2 * a + 2, ts(i, P)],
                                        rhs=E8ns[:, 2 * a : 2 * a + 2, :],
                                        start=(a == 0),
                                        stop=(a == NPAIR - 1),
                                        perf_mode=DR,
                                    )

                        def osb_pair(lo, ops_pair):
                            for i, e4 in enumerate((lo, lo + 1)):
                                nc.vector.tensor_mul(
                                    out=osf[:, i, :], in0=ops_pair[i], in1=rz
                                )
                                # deferred v-bias: osb = PV/Z + bv2c (exact
                                # since softmax weights sum to 1)
                                nc.gpsimd.tensor_scalar_add(
                                    out=osb[:, e4, :],
                                    in0=osf[:, i, :],
                                    scalar1=bv2c_t[:, e4 : e4 + 1],
                                )

                        def pv_sweep2(ops2):
                            for a in range(NPAIR):
                                for i, e4 in enumerate((2, 3)):
                                    nc.tensor.matmul(
                                        ops2[i],
                                        lhsT=vT_t[:, 2 * a : 2 * a + 2, ts(e4, P)],
                                        rhs=E8ns[:, 2 * a : 2 * a + 2, :],
                                        start=(a == 0),
                                        stop=(a == NPAIR - 1),
                                        perf_mode=DR,
                                    )

                        if s2pool is not None:
                            # last ns: fresh banks -> sweep 2 need not wait
                            # for osb01 to release the ozp ring
                            ops2 = [
                                s2pool.tile(
                                    [P, 512], F32, tag="o2", name=f"o{ns}s2_{e4}"
                                )
                                for e4 in (2, 3)
                            ]
                            pv_sweep2(ops2)
                            nc.vector.reciprocal(out=rz, in_=zps)
                            osb_pair(0, ops)
                        else:
                            nc.vector.reciprocal(out=rz, in_=zps)
                            osb_pair(0, ops)
                            # PV sweep 2: e4 2,3 from retained E8 (reuses the
                            # two ozp banks osb01 just released)
                            ops2 = [
                                ozp.tile(
                                    [P, 512], F32, tag="oz", name=f"o{ns}s2_{e4}"
                                )
                                for e4 in (2, 3)
                            ]
                            pv_sweep2(ops2)
                        osb_pair(2, ops2)
                        ppool = s2pool if s2pool is not None else ppp_cell[0]
                        for oj in range(CCH):
                            pp = ppool.tile(
                                [P, 512], F32, tag="pp" if s2pool is None else "o2",
                                name=f"pp{ns}_{oj}",
                            )
                            for c2 in range(2):
                                nc.tensor.matmul(
                                    pp,
                                    lhsT=w8o[:, 2 * c2 : 2 * c2 + 2, ts(oj, P)],
                                    rhs=osb[:, 2 * c2 : 2 * c2 + 2, :],
                                    start=(c2 == 0),
                                    stop=(c2 == 1),
                                    perf_mode=DR,
                                )
                            # fp16 store: output values are ~N(0,1); the
                            # host assembles into fp32 (+~5e-4 rel err vs a
                            # 2e-2 gate) and the final DMA drain halves
                            res = finp.tile(
                                [P, 512], F16, tag="res", name=f"r{ns}_{oj}"
                            )
                            # res = pp/64 + (x + bo)
                            nc.vector.scalar_tensor_tensor(
                                out=res,
                                in0=pp,
                                scalar=INV64,
                                in1=xb_t[:, oj, ts(ns, 512)],
                                op0=OP.mult,
                                op1=OP.add,
                            )
                            nc.sync.dma_start(
                                out=out_p[oj, :, ts(ns, 512)], in_=res
                            )

                    # ---- region 1: projections + ns=0 m-loop ----
                    E8_0 = e8bp.tile([P, MC, 512], FP8, tag="E8", name="E8_0")

                    with (
                        tc.tile_pool(name="bpsum", bufs=4, space="PSUM") as bpp,
                        tc.tile_pool(name="s0p", bufs=2, space="PSUM") as s0p,
                    ):

                        def kproj(nb):
                            for oj in range(CCH):
                                kp = bpp.tile([P, 512], F32, tag="pj")
                                for c2 in range(2):
                                    nc.tensor.matmul(
                                        kp,
                                        lhsT=w8k[:, 2 * c2 : 2 * c2 + 2, ts(oj, P)],
                                        rhs=xb16[:, 2 * c2 : 2 * c2 + 2, ts(nb, 512)],
                                        start=(c2 == 0),
                                        stop=(c2 == 1),
                                        perf_mode=DR,
                                    )
                                # Exp and Identity share an act table, so
                                # ScalarE epilogues interleave with the exp
                                # stream at no table-reload cost
                                proj_epi(
                                    k_t[:, oj, ts(nb, 512)],
                                    kp,
                                    bk2_t[:, oj : oj + 1],
                                    on_act=((nb < 2 and oj % 2 == 0) or (nb >= 2 and oj % 2 == 0)),
                                )

                        def vproj(nb):
                            for mj in range(4):
                                vp = bpp.tile([P, 512], F32, tag="pj")
                                for c2 in range(2):
                                    nc.tensor.matmul(
                                        vp,
                                        lhsT=xb16[
                                            :, 2 * c2 : 2 * c2 + 2, ts(nb * 4 + mj, P)
                                        ],
                                        rhs=w8v[:, 2 * c2 : 2 * c2 + 2, :],
                                        start=(c2 == 0),
                                        stop=(c2 == 1),
                                        perf_mode=DR,
                                    )
                                proj_epi(
                                    vT_t[:, nb * 4 + mj, :],
                                    vp,
                                    None,
                                    on_act=((nb < 2 and mj % 2 == 0) or (nb >= 2 and mj == 0)),
                                )

                        def qproj(nb):
                            for oj in range(CCH):
                                qp = bpp.tile([P, 512], F32, tag="pj")
                                for c2 in range(2):
                                    nc.tensor.matmul(
                                        qp,
                                        lhsT=w8q[:, 2 * c2 : 2 * c2 + 2, ts(oj, P)],
                                        rhs=xb16[:, 2 * c2 : 2 * c2 + 2, ts(nb, 512)],
                                        start=(c2 == 0),
                                        stop=(c2 == 1),
                                        perf_mode=DR,
                                    )
                                proj_epi(
                                    q_t[:, oj, ts(nb, 512)],
                                    qp,
                                    bq2_t[:, oj : oj + 1],
                                    on_act=(oj % 2 == 1 if nb == 0 else oj == 3),
                                )

                        # ns=0 S/exp/PV machinery riding the projection
                        # loop, in exp-pairs like region 2 (no Z: its PSUM
                        # bank is spent on the pair tiles; Z0 is recomputed
                        # from E8_0 in finalize)
                        s0_pend = []  # (pair, psum_tile)
                        s0_next = [0]

                        def s0_fill():
                            a = s0_next[0]
                            sp = s0p.tile([P, 2, 512], F32, tag="s", name=f"s0_{a}")
                            for j in range(2):
                                for c2 in range(2):
                                    nc.tensor.matmul(
                                        sp[:, j, :],
                                        lhsT=k_t[
                                            :, 2 * c2 : 2 * c2 + 2, ts(2 * a + j, P)
                                        ],
                                        rhs=q_t[:, 2 * c2 : 2 * c2 + 2, ts(0, 512)],
                                        start=(c2 == 0),
                                        stop=(c2 == 1),
                                        perf_mode=DR,
                                    )
                            s0_pend.append((a, sp))
                            s0_next[0] += 1

                        def s0_drain():
                            a, sp = s0_pend.pop(0)
                            nc.scalar.activation(
                                out=E8_0[:, 2 * a : 2 * a + 2, :],
                                in_=sp,
                                func=AF.Exp,
                                bias=zero_t,
                                scale=SCALE,
                            )

                        kproj(0)
                        vproj(0)
                        qproj(0)
                        for nb in range(1, NB):
                            # S fills for the previous nb's key pairs go
                            # FIRST (their k tiles landed last block), so
                            # the exp stream isn't queued behind this
                            # block's projection matmuls on PE
                            for _ in range(2):
                                s0_fill()
                                if len(s0_pend) >= 2:
                                    s0_drain()
                            kproj(nb)
                            vproj(nb)
                            if nb in (2, 3):
                                qproj(nb - 1)
                        for _ in range(2):
                            s0_fill()
                            if len(s0_pend) >= 2:
                                s0_drain()
                        while s0_pend:
                            s0_drain()

                    # ---- region 2: ns=1..3 paired-exp m-loops with the ----
                    # ---- previous ns's finalize rolled in               ----
                    with (
                        tc.tile_pool(name="ppp", bufs=1, space="PSUM") as ppp,
                        tc.tile_pool(name="zpsp", bufs=1, space="PSUM") as zpsp,
                        tc.tile_pool(name="ozp", bufs=2, space="PSUM") as ozp,
                    ):
                        ppp_cell[0] = ppp
                        zps_cell[0] = zpsp
                        ozp_cell[0] = ozp
                        with tc.tile_pool(name="spp", bufs=2, space="PSUM") as spp:
                            pend = (0, None, None, E8_0)

                            def s_fill_pair(ns, a):
                                sp = spp.tile(
                                    [P, 2, 512], F32, tag="s", name=f"s{ns}_{a}"
                                )
                                for j in range(2):
                                    for c2 in range(2):
                                        nc.tensor.matmul(
                                            sp[:, j, :],
                                            lhsT=k_t[
                                                :, 2 * c2 : 2 * c2 + 2,
                                                ts(2 * a + j, P),
                                            ],
                                            rhs=q_t[
                                                :, 2 * c2 : 2 * c2 + 2, ts(ns, 512)
                                            ],
                                            start=(c2 == 0),
                                            stop=(c2 == 1),
                                            perf_mode=DR,
                                        )
                                return sp

                            def exp_pair(E8ns, a, sp):
                                nc.scalar.activation(
                                    out=E8ns[:, 2 * a : 2 * a + 2, :],
                                    in_=sp,
                                    func=AF.Exp,
                                    bias=zero_t,
                                    scale=SCALE,
                                )

                            def qproj3():
                                # q block 3 deferred out of region 1: its
                                # psums borrow the out-proj bank, which is
                                # idle between finalizes
                                for oj in range(CCH):
                                    qp = ppp.tile(
                                        [P, 512], F32, tag="pp", name=f"q3_{oj}"
                                    )
                                    for c2 in range(2):
                                        nc.tensor.matmul(
                                            qp,
                                            lhsT=w8q[
                                                :, 2 * c2 : 2 * c2 + 2, ts(oj, P)
                                            ],
                                            rhs=xb16[
                                                :, 2 * c2 : 2 * c2 + 2, ts(3, 512)
                                            ],
                                            start=(c2 == 0),
                                            stop=(c2 == 1),
                                            perf_mode=DR,
                                        )
                                    proj_epi(
                                        q_t[:, oj, ts(3, 512)],
                                        qp,
                                        bq2_t[:, oj : oj + 1],
                                        on_act=False,
                                    )

                            # LOOK=16: the whole S/exp stream of an ns is
                            # emitted before the previous ns's finalize, so
                            # ScalarE's instruction order never waits on
                            # finalize PE work. The last two drains of each
                            # ns are CARRIED into the next ns block, after
                            # its first two S fills:
                            # PE then refills the just-freed S banks before
                            # the previous ns's PV tail, so ScalarE crosses
                            # the boundary without a gap.
                            carry = []  # (exp-args, pv-args)
                            LOOK = 16
                            for ns in range(1, NBH):
                                E8ns = e8bp.tile(
                                    [P, MC, 512], FP8, tag="E8", name=f"E8_{ns}"
                                )
                                sps = {}
                                for a in range(2):
                                    sps[a] = s_fill_pair(ns, a)
                                for (E8p, ap, spp_), (opsp, zpsp_) in carry:
                                    exp_pair(E8p, ap, spp_)
                                    rhs = E8p[:, 2 * ap : 2 * ap + 2, :]
                                    mk_pv1(opsp, ap, rhs)
                                    mk_z(zpsp_, ap, rhs)
                                carry = []
                                for a in range(2, LOOK):
                                    sps[a] = s_fill_pair(ns, a)
                                    exp_pair(E8ns, a - 2, sps.pop(a - 2))
                                finalize(*pend)
                                if ns == 1:
                                    qproj3()
                                ops = [
                                    ozp.tile(
                                        [P, 512], F32, tag="oz", name=f"o{ns}_{e4}"
                                    )
                                    for e4 in range(2)
                                ]
                                zps = zpsp.tile(
                                    [P, 512], F32, tag="z", name=f"z{ns}"
                                )
                                for a in range(LOOK - 2):
                                    rhs = E8ns[:, 2 * a : 2 * a + 2, :]
                                    mk_pv1(ops, a, rhs)
                                    mk_z(zps, a, rhs)
                                for a in range(LOOK, NPAIR):
                                    sps[a] = s_fill_pair(ns, a)
                                    ap = a - 2
                                    exp_pair(E8ns, ap, sps.pop(ap))
                                    rhs = E8ns[:, 2 * ap : 2 * ap + 2, :]
                                    mk_pv1(ops, ap, rhs)
                                    mk_z(zps, ap, rhs)
                                for a in (NPAIR - 2, NPAIR - 1):
                                    carry.append(
                                        ((E8ns, a, sps.pop(a)), (ops, zps))
                                    )
                                pend = (ns, ops, zps, E8ns)
                            # drain the final carry, then the last finalize
                            for (E8p, ap, spp_), (opsp, zpsp_) in carry:
                                exp_pair(E8p, ap, spp_)
                                rhs = E8p[:, 2 * ap : 2 * ap + 2, :]
                                mk_pv1(opsp, ap, rhs)
                                mk_z(zpsp_, ap, rhs)
                        with tc.tile_pool(name="s2p", bufs=2, space="PSUM") as s2pool:
                            finalize(*pend, s2pool=s2pool)

    nc.compile()
    return nc


def _prep_consts(inputs):
    bf = ml_dtypes.bfloat16

    def wt(w):
        # w: [o, c] -> lhsT layout [c, o] chunked by c: [P, CCH, C]
        return np.ascontiguousarray(
            w.T.reshape(CCH, P, C).transpose(1, 0, 2)
        )

    def colvec(b):
        return np.ascontiguousarray(b.reshape(CCH, P).T).astype(np.float32)

    ind = np.zeros((P, 8), np.float32)
    ind[np.arange(P), np.arange(P) // 16] = 1.0 / 16.0
    ind2 = np.zeros((8, P), np.float32)
    ind2[np.arange(P) // 16, np.arange(P)] = 1.0
    gmat = np.ascontiguousarray(ind @ ind2)

    return {
        "wqT": wt(np.asarray(inputs["wq"], np.float32)).astype(bf),
        "wkT": wt(np.asarray(inputs["wk"], np.float32)).astype(bf),
        "wvT": wt(np.asarray(inputs["wv"], np.float32)).astype(bf),
        "wo8": (wt(np.asarray(inputs["wo"], np.float32)) * 64.0).astype(
            ml_dtypes.float8_e4m3
        ),
        "bq": colvec(np.asarray(inputs["bq"], np.float32)),
        "bk": colvec(np.asarray(inputs["bk"], np.float32)),
        "bo": colvec(np.asarray(inputs["bo"], np.float32)),
        "bvc": colvec(np.asarray(inputs["bv"], np.float32)),
        "gnw": colvec(np.asarray(inputs["gn_w"], np.float32)),
        "gnb": colvec(np.asarray(inputs["gn_b"], np.float32)),
        "ones8": np.ones((P, 2, P), ml_dtypes.float8_e4m3),
        "gmat": gmat,
    }


def kernel(**inputs):
    global LAST_EXEC_TIME_NS, _CACHED_NC, _last_in_maps
    x = np.asarray(inputs["x"], np.float32)  # [4, 512, 64, 64]
    B = x.shape[0]
    assert x.shape == (4, C, 64, 64)

    if _CACHED_NC is None:
        _CACHED_NC = build_nc()
    nc = _CACHED_NC

    consts = _prep_consts(inputs)
    xf = np.ascontiguousarray(x.reshape(B, CCH, P, N))

    in_maps = []
    for core in range(8):
        b, half = core // 2, core % 2
        m = dict(consts)
        if half == 0:
            m["xr"] = xf[b].astype(np.float16)
        else:
            # rotate columns so this core's 2048 query columns come first
            m["xr"] = np.ascontiguousarray(
                np.concatenate(
                    [xf[b][:, :, NHALF:], xf[b][:, :, :NHALF]], axis=2
                )
            ).astype(np.float16)
        in_maps.append(m)

    _last_in_maps = in_maps
    res = run_bass_kernel_spmd(nc, in_maps, core_ids=list(range(8)))
    LAST_EXEC_TIME_NS = res.exec_time_ns

    out = np.empty((B, C, N), np.float32)
    for core in range(8):
        b, half = core // 2, core % 2
        out[b, :, half * NHALF : (half + 1) * NHALF] = (
            res.results[core]["out"].reshape(C, NHALF)
        )
    return out.reshape(B, C, 64, 64)

